# revision 23
# baseline (speedup 1.0000x reference)
"""Multi-head causal attention (B=4, L=2048, D=1024, H=16) on 8 trn2 cores.

Sharding: (batch, head-group) grid — core c handles batch c//2, heads
(c%2)*8..(c%2)*8+8.  Each core projects Q/K/V for its 8 heads, runs causal
attention, and computes a partial output projection; the host sums the two
head-group partials per batch.

Schedule notes: the kernel is PE-bound overall and ACT(EXP)-bound inside
the attention stream, so the schedule keeps both dense:
  - single-(head-pair) sweeps: pv accumulators use 2 PSUM banks (pool of 3),
    one bank is dedicated to Wo matmuls so they interleave into sweeps;
  - causal tri-mask is a 0/1 DVE multiply on the exp'd scores (no PE mask
    matmuls); ACT does EXP only, all evacuations ride DVE;
  - pv normalization: DVE evac -> tiny DMA of the denominator row to
    partition 0 -> fast reciprocal -> GPSIMD broadcast -> DVE muls, all
    deferred off the critical path via a strict-FIFO due-counter queue
    (trace order is the dependency order — never reorder entries);
  - projections are per-head-pair filler closures (8 matmuls) drained
    between attention chunks so proj fills PE slack and EXP never runs dry;
  - inputs are host-relayouted to [128, t, d, 512] blocks (8KB contiguous
    per-partition DMA lines, one descriptor per block), issued in first-use
    order (wk, xk0, wq, xq0, wv, xv0);
  - ~120 warmup matmuls cover the initial DMA window and tail keepwarm
    matmuls (gated on the last ex tile) bridge the final norm chain, both
    to keep the HAM clock gate at 2.4GHz;
  - the last pair's partition shift uses a PE identity matmul + DVE evac
    (a tiny SBUF->SBUF DMA costs ~2us latency at the tail).

Per-core layouts (host prepares transposed inputs so every matmul contracts
over the partition dim):
  xq_t/xk_t/xv_t [D, L]   : x.T            (rhs / lhsT of projections)
  wq_t/wk_t/wv_t [D, 512] : W_slice.T      (wq pre-scaled by 1/sqrt(dh))
  wo_t           [512, D] : Wo_slice.T
  qT/kT pair tiles [128, L]: rows 0-63 head 2p, 64-127 head 2p+1 (dh on P)
  v_aug [128, 8, 65]      : per 128-token chunk; [:, h, 0:64]=V, [:, h, 64]=key mask
  scores ST [k(P), q(F)]  : transposed scores -> softmax sum via matmul's
                            extra mask column (pv row 64), no P-transposes.
"""

import math
from contextlib import ExitStack

import numpy as np

import concourse.bass as bass
import concourse.tile as tile
from concourse import bacc, mybir
from concourse import bass_utils

D = 1024  # model dim
HG = 512  # head dims per core (8 heads x 64)
NH = 8    # heads per core
DH = 64
NPAIR = 4  # head pairs per core

F32 = mybir.dt.float32
BF16 = mybir.dt.bfloat16
EXP = mybir.ActivationFunctionType.Exp


def build(L=2048):
    TQ = L // 512    # 512-token q-blocks
    T16 = L // 128   # 128-token chunks
    DCH = D // 128   # contraction chunks for projections
    nc = bacc.Bacc("TRN2", target_bir_lowering=False, debug=False, num_devices=8)

    xq = nc.dram_tensor("xq_r", [128, TQ, DCH, 512], BF16, kind="ExternalInput").ap()
    xk = nc.dram_tensor("xk_r", [128, TQ, DCH, 512], BF16, kind="ExternalInput").ap()
    xv = nc.dram_tensor("xv_r", [128, TQ, DCH, 512], BF16, kind="ExternalInput").ap()
    wq = nc.dram_tensor("wq_r", [128, DCH, HG], BF16, kind="ExternalInput").ap()
    wk = nc.dram_tensor("wk_r", [128, DCH, HG], BF16, kind="ExternalInput").ap()
    wv = nc.dram_tensor("wv_r", [128, DCH, HG], BF16, kind="ExternalInput").ap()
    wo = nc.dram_tensor("wo_t", [HG, D], BF16, kind="ExternalInput").ap()
    mcol = nc.dram_tensor("maskcol", [128, (L // 128) * NH], F32, kind="ExternalInput").ap()
    trim = nc.dram_tensor("trimask", [128, 2 * 128], BF16, kind="ExternalInput").ap()
    iden = nc.dram_tensor("ident64", [DH, DH], BF16, kind="ExternalInput").ap()
    outp = nc.dram_tensor("outp", [L, D], BF16, kind="ExternalOutput").ap()

    with ExitStack() as ctx:
        tc = ctx.enter_context(tile.TileContext(nc))

        # ---- persistent tiles ----
        singles = ctx.enter_context(tc.tile_pool(name="singles", bufs=1))
        qT = [singles.tile([128, L], BF16, tag=f"qT{p}", name=f"qT{p}") for p in range(NPAIR)]
        kT = [singles.tile([128, L], BF16, tag=f"kT{p}", name=f"kT{p}") for p in range(NPAIR)]
        vaug = [singles.tile([128, NH, DH + 1], BF16, tag=f"vaug{t}", name=f"vaug{t}") for t in range(T16)]
        ctxT = [singles.tile([128, L], BF16, tag=f"ctxT{p}", name=f"ctxT{p}") for p in range(NPAIR)]
        mc_sb = singles.tile([128, T16, NH], F32, tag="mc")
        tri_sb = singles.tile([128, 2, 128], BF16, tag="tri")
        id_sb = singles.tile([DH, DH], BF16, tag="id64")

        nc.sync.dma_start(out=mc_sb, in_=mcol.rearrange("p (t h) -> p t h", h=NH))
        nc.sync.dma_start(out=tri_sb, in_=trim.rearrange("p (u q) -> p u q", u=2))
        nc.sync.dma_start(out=id_sb, in_=iden)

        with (
            tc.tile_pool(name="xt", bufs=5) as xtp,
            tc.tile_pool(name="w", bufs=3) as wp,
            tc.tile_pool(name="stp", bufs=2, space="PSUM") as stp,     # 4 banks (scores + proj)
            tc.tile_pool(name="pvp", bufs=3, space="PSUM") as pvp,     # 3 banks (pv accum)
            tc.tile_pool(name="wops", bufs=1, space="PSUM") as wops,   # 1 bank (wo + warmup)
            tc.tile_pool(name="expp", bufs=6) as expp,
            tc.tile_pool(name="pvsb", bufs=6) as pvsbp,
            tc.tile_pool(name="dsb", bufs=2) as dsbp,
            tc.tile_pool(name="rcb", bufs=2) as rcbp,
            tc.tile_pool(name="bcs", bufs=4) as bcsp,
            tc.tile_pool(name="tbp", bufs=3) as tbp,
            tc.tile_pool(name="wop", bufs=NPAIR) as wop,
            tc.tile_pool(name="outp_sb", bufs=3) as outsb,
        ):
            # Warm the PE clock (HAM) while the first input DMAs land
            # (~12us of dummy matmuls; vary operands/outputs so nothing
            # collapses into zero-duration issues).
            wu = singles.tile([128, 512], BF16, tag="warm")
            nc.vector.memset(wu, 0.0)
            wups = wops.tile([128, 512], F32, tag="wo", name="wupstile")
            for i in range(110):
                nc.tensor.matmul(
                    wups[:, (i % 2) * 256:(i % 2) * 256 + 256],
                    lhsT=wu[:, (i % 4) * 128:(i % 4) * 128 + 128],
                    rhs=wu[:, 0:256] if i % 2 == 0 else wu[:, 256:512],
                    start=True,
                    stop=True,
                )

            # ---- deferred-filler machinery: (due_chunk, fn) queue ----
            chunk_ctr = [0]
            pending = []

            def defer(margin, fn):
                pending.append([chunk_ctr[0] + margin, fn])

            def drain(limit=3):
                # strict FIFO: trace order IS the dependency order for
                # read-after-write pairs queued through here (e.g. norm_b
                # writes ctxT, wo_group reads it) — never reorder.
                n = 0
                while pending and pending[0][0] <= chunk_ctr[0] and n < limit:
                    pending.pop(0)[1]()
                    n += 1

            def drain_all():
                while pending:
                    pending.pop(0)[1]()

            # ---- projections ----
            def load_w(wdram, split=False):
                wt = wp.tile([128, DCH, HG], BF16, tag="w", name="wtile")
                if split:
                    nc.sync.dma_start(out=wt[:, :, 0:HG // 2], in_=wdram[:, :, 0:HG // 2])
                    nc.sync.dma_start(out=wt[:, :, HG // 2:HG], in_=wdram[:, :, HG // 2:HG])
                else:
                    nc.sync.dma_start(out=wt, in_=wdram)
                return wt

            def load_xts(xdram, t):
                xt = xtp.tile([128, DCH, 512], BF16, tag="xt", name="xtile")
                nc.sync.dma_start(out=xt, in_=xdram[:, t, :, :])
                return xt

            def proj_group_T(wt, xt, dst, m, t):
                # dst[m][:, t*512:+512] = (W.T chunk m).T @ xT
                ps = stp.tile([128, 512], F32, tag="st", name="psproj")
                for d in range(DCH):
                    nc.tensor.matmul(
                        ps,
                        lhsT=wt[:, d, m * 128:(m + 1) * 128],
                        rhs=xt[:, d, :],
                        start=(d == 0),
                        stop=(d == DCH - 1),
                    )
                nc.vector.tensor_copy(dst[m][:, t * 512:(t + 1) * 512], ps)

            def proj_group_V(wt, xt, t, s):
                # v_aug[t*4+s][:, h, 0:64] = (x @ Wv.T)[tok chunk, head h], masked
                t16 = t * 4 + s
                ps = stp.tile([128, 512], F32, tag="st", name="psv")
                for d in range(DCH):
                    nc.tensor.matmul(
                        ps,
                        lhsT=xt[:, d, s * 128:(s + 1) * 128],
                        rhs=wt[:, d, :],
                        start=(d == 0),
                        stop=(d == DCH - 1),
                    )
                nc.vector.tensor_scalar_mul(
                    vaug[t16][:, :, 0:DH],
                    ps.rearrange("p (h e) -> p h e", h=NH),
                    mc_sb[:, t16, 0:1],
                )
                nc.vector.tensor_copy(
                    vaug[t16][:, :, DH:DH + 1],
                    mc_sb[:, t16:t16 + 1, :],
                )

            wk_t = load_w(wk, split=True)
            wq_t = load_w(wq)
            wv_t = load_w(wv)

            # ---- wo output projection for one 128-token chunk, one half ----
            ot_tiles = {}
            wo_done = {}

            def wo_group(t16, oh, pool, tag):
                def fn():
                    ps = pool.tile([128, 512], F32, tag=tag, name="potile")
                    for c in range(NPAIR):
                        nc.tensor.matmul(
                            ps,
                            lhsT=ctxT[c][:, t16 * 128:(t16 + 1) * 128],
                            rhs=wo_sb[c][:, oh * 512:(oh + 1) * 512],
                            start=(c == 0),
                            stop=(c == NPAIR - 1),
                        )
                    ot = ot_tiles.get(t16)
                    if ot is None:
                        ot = outsb.tile([128, D], BF16, tag="ot", name="ottile")
                        ot_tiles[t16] = ot
                    nc.vector.tensor_copy(ot[:, oh * 512:(oh + 1) * 512], ps)
                    done = wo_done.setdefault(t16, [])
                    done.append(oh)
                    if len(done) == 2:
                        nc.sync.dma_start(
                            out=outp[t16 * 128:(t16 + 1) * 128, :], in_=ot
                        )
                        del ot_tiles[t16]
                return fn

            last_ex = [None]

            # ---- one attention sweep: q-block qb, head pair p ----
            def sweep(qb, p):
                nkc = 4 * (qb + 1)
                pv = [
                    pvp.tile([DH + 1, 512], F32, tag="pv", name="pvtile")
                    for _ in range(2)
                ]

                def issue_pv(kc, ex, off):
                    for ph in range(2):
                        nc.tensor.matmul(
                            pv[ph][:, off:512],
                            lhsT=vaug[kc][:, 2 * p + ph, :],
                            rhs=ex[:, ph, off:512],
                            start=(kc == 0),
                            stop=(kc == nkc - 1),
                        )

                prev = None
                last_ex[0] = None
                for kc in range(nkc):
                    j = kc - 4 * qb  # >=0 -> diagonal 512-block
                    off = j * 128 if j >= 0 else 0
                    # PV of the previous chunk first: its ex is ready, so a
                    # score matmul briefly blocked on an st slot can't
                    # head-block it in the PE queue
                    if prev is not None:
                        issue_pv(*prev)
                        prev = None
                    st = stp.tile([128, 2, 512], F32, tag="st", name="sttile")
                    for ph in range(2):
                        nc.tensor.matmul(
                            st[:, ph, off:512],
                            lhsT=kT[p][ph * DH:(ph + 1) * DH,
                                       kc * 128:(kc + 1) * 128],
                            rhs=qT[p][ph * DH:(ph + 1) * DH,
                                      qb * 512 + off:(qb + 1) * 512],
                            start=True,
                            stop=True,
                        )
                    ex = expp.tile([128, 2, 512], BF16, tag="expst", name="extile")
                    nc.scalar.activation(
                        out=ex[:, :, off:512], in_=st[:, :, off:512], func=EXP
                    )
                    if j >= 0:
                        # causal mask: zero the strictly-upper part of the
                        # 128x128 diagonal block (0/1 multiply on DVE)
                        nc.vector.tensor_mul(
                            ex[:, :, off:off + 128],
                            ex[:, :, off:off + 128],
                            tri_sb,
                        )
                    prev = (kc, ex, off)
                    last_ex[0] = ex
                    chunk_ctr[0] += 1
                    drain()
                issue_pv(*prev)

                # evacuate pv -> SBUF promptly (frees the 2 banks); move the
                # mask row to partition 0 with a tiny DMA for the reciprocal.
                pvs = [
                    pvsbp.tile([DH + 1, 512], F32, tag="pvs", name="pvstile")
                    for _ in range(2)
                ]
                ds = dsbp.tile([1, 2, 512], F32, tag="ds", name="dstile")
                if qb == TQ - 1 and p == NPAIR - 1:
                    # tail: denominator rows first so their DMA launches
                    # ~0.7us earlier; bulk evacuation follows
                    for ph in range(2):
                        nc.vector.tensor_copy(
                            pvs[ph][DH:DH + 1, :], pv[ph][DH:DH + 1, :]
                        )
                        nc.sync.dma_start(
                            out=ds[0:1, ph, :], in_=pvs[ph][DH:DH + 1, :]
                        )
                    for ph in range(2):
                        nc.vector.tensor_copy(pvs[ph][0:DH, :], pv[ph][0:DH, :])
                else:
                    for ph in range(2):
                        nc.vector.tensor_copy(pvs[ph], pv[ph])
                        nc.sync.dma_start(out=ds[0:1, ph, :], in_=pvs[ph][DH:DH + 1, :])

                bcs = [None, None]

                def norm_a():
                    rc = rcbp.tile([1, 2, 512], F32, tag="rc", name="rctile")
                    nc.vector.reciprocal_approx_fast(
                        out=rc[0:1, :, :], in_=ds[0:1, :, :]
                    )
                    for ph in range(2):
                        bcs[ph] = bcsp.tile([DH, 512], F32, tag="bcs", name="bcstile")
                        nc.gpsimd.partition_broadcast(
                            bcs[ph], rc[0:1, ph, :], channels=DH
                        )

                def norm_b():
                    nc.vector.tensor_mul(
                        ctxT[p][0:DH, qb * 512:(qb + 1) * 512],
                        pvs[0][0:DH, :],
                        bcs[0],
                    )
                    tb = tbp.tile([DH, 512], BF16, tag="tb", name="tbtile")
                    nc.vector.tensor_mul(tb, pvs[1][0:DH, :], bcs[1])
                    if qb == TQ - 1 and p == NPAIR - 1:
                        # tail: partition shift via PE identity matmul + DVE
                        # evac (a tiny SBUF->SBUF DMA costs ~2us of latency
                        # here and the idle re-throttles the PE clock)
                        ps = stp.tile([128, 512], F32, tag="st", name="shifttile")
                        nc.tensor.matmul(
                            ps[DH:128, :], lhsT=id_sb, rhs=tb, start=True, stop=True
                        )
                        nc.vector.tensor_copy(
                            ctxT[p][DH:128, qb * 512:(qb + 1) * 512], ps[DH:128, :]
                        )
                    else:
                        # partition shift rows 0-63 -> 64-127 via DMA
                        nc.sync.dma_start(
                            out=ctxT[p][DH:128, qb * 512:(qb + 1) * 512],
                            in_=tb,
                        )

                if qb == TQ - 1:
                    defer(1, norm_a)
                    defer(2, norm_b)
                else:
                    defer(2, norm_a)
                    defer(4, norm_b)

            # ---- main schedule ----
            xk0 = load_xts(xk, 0)
            for m in range(NPAIR):
                proj_group_T(wk_t, xk0, kT, m, 0)
            xq0 = load_xts(xq, 0)
            for m in range(NPAIR):
                proj_group_T(wq_t, xq0, qT, m, 0)
            xv0 = load_xts(xv, 0)
            wo_sb = [wop.tile([128, D], BF16, tag="wo", name="wotile") for _ in range(NPAIR)]
            for c in range(NPAIR):
                nc.sync.dma_start(out=wo_sb[c], in_=wo[c * 128:(c + 1) * 128, :])
            for s in range(4):
                defer(1 + s, (lambda s=s: proj_group_V(wv_t, xv0, 0, s)))

            for t in range(TQ):
                qb = t
                nchunks = 4 * 4 * (qb + 1)
                # queue next t-block's projections as fillers spread over
                # this block's sweeps (x DMAs issued inside the closures)
                if t + 1 < TQ:
                    xnext = {}
                    step = max(1, nchunks // 16)

                    def mk_load(xdram, key, t1=t + 1, xn=xnext):
                        def fn():
                            xn[key] = load_xts(xdram, t1)
                        return fn

                    def mk_T(wt, key, dst, m, t1=t + 1, xn=xnext):
                        def fn():
                            proj_group_T(wt, xn[key], dst, m, t1)
                        return fn

                    def mk_V(wt, key, s, t1=t + 1, xn=xnext):
                        def fn():
                            proj_group_V(wt, xn[key], t1, s)
                        return fn

                    fill = [mk_load(xk, "k")]
                    fill += [mk_T(wk_t, "k", kT, m) for m in range(NPAIR)]
                    fill += [mk_load(xq, "q")]
                    fill += [mk_T(wq_t, "q", qT, m) for m in range(NPAIR)]
                    fill += [mk_load(xv, "v")]
                    fill += [mk_V(wv_t, "v", s) for s in range(4)]
                    for idx, fn in enumerate(fill):
                        defer(1 + idx * step, fn)

                for p in range(NPAIR):
                    sweep(qb, p)

                # queue this q-block's output projection
                for i, t16 in enumerate(range(4 * qb, 4 * qb + 4)):
                    for oh in range(2):
                        if qb == TQ - 1:
                            pool, tag = [(wops, "wo"), (stp, "st")][(2 * i + oh) % 2]
                            due = 1 + i
                        else:
                            pool, tag = wops, "wo"
                            due = 2 + 2 * i + oh
                        defer(due, wo_group(t16, oh, pool, tag))

            # keep the PE clock warm across the final norm chain (the
            # ~5us idle otherwise re-throttles HAM and the tail wo matmuls
            # run at half clock); reading the last ex tile pins these to
            # the tail (they'd float to the front with no dependencies)
            kw = pvp.tile([DH + 1, 512], F32, tag="pv", name="kwtile")
            lex = last_ex[0]
            for i in range(56):
                nc.tensor.matmul(
                    kw[0:DH, (i % 2) * 256:(i % 2) * 256 + 256],
                    lhsT=wu[0:DH, 0:DH],
                    rhs=lex[0:DH, i % 2, 256:512],
                    start=True,
                    stop=True,
                )
            drain_all()

    nc.compile()
    return nc


_CACHE = {}


def _get_nc(L):
    if L not in _CACHE:
        _CACHE[L] = build(L)
    return _CACHE[L]


def make_in_maps(query, key, value, attention_mask, Wq, Wk, Wv, Wo):
    import ml_dtypes

    B, L, _ = query.shape
    scale = np.float32(1.0 / math.sqrt(DH))
    bf = lambda a: np.ascontiguousarray(np.asarray(a, np.float32)).astype(
        ml_dtypes.bfloat16
    )
    TQ, DCH = L // 512, D // 128
    # [p, t, d, l] = x.T[d*128+p, t*512+l] -> 8KB contiguous lines per block
    def relayout(x):
        xt = np.asarray(x, np.float32).T.reshape(DCH, 128, TQ, 512)
        return bf(np.ascontiguousarray(xt.transpose(1, 2, 0, 3)))

    xqT = [relayout(query[b]) for b in range(B)]
    xkT = [relayout(key[b]) for b in range(B)]
    xvT = [relayout(value[b]) for b in range(B)]
    kk, qq = np.meshgrid(np.arange(128), np.arange(128), indexing="ij")
    tri01 = np.where(kk <= qq, np.float32(1.0), np.float32(0.0)).astype(np.float32)
    tri2 = np.ascontiguousarray(np.concatenate([tri01, tri01], axis=1))
    in_maps = []
    for core in range(2 * B):
        b, hg = divmod(core, 2)
        sl = slice(hg * HG, (hg + 1) * HG)
        m2 = np.asarray(attention_mask[b]).astype(np.float32).reshape(-1, 128).T
        mc = np.ascontiguousarray(
            np.repeat(m2[:, :, None], NH, 2).reshape(128, -1), dtype=np.float32
        )
        def wrelayout(wt):
            # [p, d, :] = W_slice.T[d*128+p, :]
            return bf(np.ascontiguousarray(
                np.asarray(wt, np.float32).reshape(DCH, 128, HG).transpose(1, 0, 2)
            ))

        in_maps.append({
            "xq_r": xqT[b],
            "xk_r": xkT[b],
            "xv_r": xvT[b],
            "wq_r": wrelayout(np.asarray(Wq, np.float32)[sl, :].T * scale),
            "wk_r": wrelayout(np.asarray(Wk, np.float32)[sl, :].T),
            "wv_r": wrelayout(np.asarray(Wv, np.float32)[sl, :].T),
            "wo_t": bf(np.asarray(Wo, np.float32)[:, sl].T),
            "maskcol": mc,
            "trimask": bf(tri2),
            "ident64": bf(np.eye(DH, dtype=np.float32)),
        })
    return in_maps


def kernel(query, key, value, attention_mask, Wq, Wk, Wv, Wo, _res_hook=None):
    B, L, D_ = query.shape
    nc = _get_nc(L)
    in_maps = make_in_maps(query, key, value, attention_mask, Wq, Wk, Wv, Wo)
    res = bass_utils.run_bass_kernel_spmd(nc, in_maps, core_ids=list(range(8)))
    if _res_hook is not None:
        _res_hook(res)
    out = np.empty((B, L, D_), np.float32)
    for b in range(B):
        out[b] = res.results[2 * b]["outp"].astype(np.float32) + res.results[
            2 * b + 1
        ]["outp"].astype(np.float32)
    return out


# revision 24
# speedup vs baseline: 1.0249x; 1.0249x over previous
"""Multi-head causal attention (B=4, L=2048, D=1024, H=16) on 8 trn2 cores.

Sharding: (batch, head-group) grid — core c handles batch c//2, heads
(c%2)*8..(c%2)*8+8.  Each core projects Q/K/V for its 8 heads, runs causal
attention, and computes a partial output projection; the host sums the two
head-group partials per batch.

Schedule notes: the kernel is PE-bound overall and ACT(EXP)-bound inside
the attention stream, so the schedule keeps both dense:
  - single-(head-pair) sweeps: pv accumulators use 2 PSUM banks (pool of 3),
    one bank is dedicated to Wo matmuls so they interleave into sweeps;
  - causal tri-mask is a 0/1 DVE multiply on the exp'd scores (no PE mask
    matmuls); ACT does EXP only, all evacuations ride DVE;
  - pv normalization: DVE evac -> tiny DMA of the denominator row to
    partition 0 -> fast reciprocal -> GPSIMD broadcast -> DVE muls, all
    deferred off the critical path via a strict-FIFO due-counter queue
    (trace order is the dependency order — never reorder entries);
  - projections are per-head-pair filler closures (8 matmuls) drained
    between attention chunks so proj fills PE slack and EXP never runs dry;
  - inputs are host-relayouted to [128, t, d, 512] blocks (8KB contiguous
    per-partition DMA lines, one descriptor per block), issued in first-use
    order (wk, xk0, wq, xq0, wv, xv0);
  - ~120 warmup matmuls cover the initial DMA window and tail keepwarm
    matmuls (gated on the last ex tile) bridge the final norm chain, both
    to keep the HAM clock gate at 2.4GHz;
  - the last pair's partition shift uses a PE identity matmul + DVE evac
    (a tiny SBUF->SBUF DMA costs ~2us latency at the tail).

Per-core layouts (host prepares transposed inputs so every matmul contracts
over the partition dim):
  xq_t/xk_t/xv_t [D, L]   : x.T            (rhs / lhsT of projections)
  wq_t/wk_t/wv_t [D, 512] : W_slice.T      (wq pre-scaled by 1/sqrt(dh))
  wo_t           [512, D] : Wo_slice.T
  qT/kT pair tiles [128, L]: rows 0-63 head 2p, 64-127 head 2p+1 (dh on P)
  v_aug [128, 8, 65]      : per 128-token chunk; [:, h, 0:64]=V, [:, h, 64]=key mask
  scores ST [k(P), q(F)]  : transposed scores -> softmax sum via matmul's
                            extra mask column (pv row 64), no P-transposes.
"""

import math
from contextlib import ExitStack

import numpy as np

import concourse.bass as bass
import concourse.tile as tile
from concourse import bacc, mybir
from concourse import bass_utils

D = 1024  # model dim
HG = 512  # head dims per core (8 heads x 64)
NH = 8    # heads per core
DH = 64
NPAIR = 4  # head pairs per core

F32 = mybir.dt.float32
BF16 = mybir.dt.bfloat16
EXP = mybir.ActivationFunctionType.Exp


def build(L=2048):
    TQ = L // 512    # 512-token q-blocks
    T16 = L // 128   # 128-token chunks
    DCH = D // 128   # contraction chunks for projections
    nc = bacc.Bacc("TRN2", target_bir_lowering=False, debug=False, num_devices=8)

    xq = nc.dram_tensor("xq_r", [128, TQ, DCH, 512], BF16, kind="ExternalInput").ap()
    xk = nc.dram_tensor("xk_r", [128, TQ, DCH, 512], BF16, kind="ExternalInput").ap()
    xv = nc.dram_tensor("xv_r", [128, TQ, DCH, 512], BF16, kind="ExternalInput").ap()
    wq = nc.dram_tensor("wq_r", [128, DCH, HG], BF16, kind="ExternalInput").ap()
    wk = nc.dram_tensor("wk_r", [128, DCH, HG], BF16, kind="ExternalInput").ap()
    wv = nc.dram_tensor("wv_r", [128, DCH, HG], BF16, kind="ExternalInput").ap()
    wo = nc.dram_tensor("wo_t", [HG, D], BF16, kind="ExternalInput").ap()
    mcol = nc.dram_tensor("maskcol", [128, (L // 128) * NH], F32, kind="ExternalInput").ap()
    trim = nc.dram_tensor("trimask", [128, 2 * 128], BF16, kind="ExternalInput").ap()
    iden = nc.dram_tensor("ident64", [DH, DH], BF16, kind="ExternalInput").ap()
    outp = nc.dram_tensor("outp", [L, D], BF16, kind="ExternalOutput").ap()

    with ExitStack() as ctx:
        tc = ctx.enter_context(tile.TileContext(nc))

        # ---- persistent tiles ----
        singles = ctx.enter_context(tc.tile_pool(name="singles", bufs=1))
        qT = [singles.tile([128, L], BF16, tag=f"qT{p}", name=f"qT{p}") for p in range(NPAIR)]
        kT = [singles.tile([128, L], BF16, tag=f"kT{p}", name=f"kT{p}") for p in range(NPAIR)]
        vaug = [singles.tile([128, NH, DH + 1], BF16, tag=f"vaug{t}", name=f"vaug{t}") for t in range(T16)]
        ctxT = [singles.tile([128, L], BF16, tag=f"ctxT{p}", name=f"ctxT{p}") for p in range(NPAIR)]
        mc_sb = singles.tile([128, T16, NH], F32, tag="mc")
        tri_sb = singles.tile([128, 2, 128], BF16, tag="tri")
        id_sb = singles.tile([DH, DH], BF16, tag="id64")

        nc.sync.dma_start(out=mc_sb, in_=mcol.rearrange("p (t h) -> p t h", h=NH))
        nc.sync.dma_start(out=tri_sb, in_=trim.rearrange("p (u q) -> p u q", u=2))
        nc.sync.dma_start(out=id_sb, in_=iden)

        with (
            tc.tile_pool(name="xt", bufs=5) as xtp,
            tc.tile_pool(name="w", bufs=3) as wp,
            tc.tile_pool(name="stp", bufs=2, space="PSUM") as stp,     # 4 banks (scores + proj)
            tc.tile_pool(name="pvp", bufs=3, space="PSUM") as pvp,     # 3 banks (pv accum)
            tc.tile_pool(name="wops", bufs=1, space="PSUM") as wops,   # 1 bank (wo + warmup)
            tc.tile_pool(name="expp", bufs=6) as expp,
            tc.tile_pool(name="pvsb", bufs=6) as pvsbp,
            tc.tile_pool(name="dsb", bufs=2) as dsbp,
            tc.tile_pool(name="rcb", bufs=2) as rcbp,
            tc.tile_pool(name="bcs", bufs=4) as bcsp,
            tc.tile_pool(name="tbp", bufs=3) as tbp,
            tc.tile_pool(name="wop", bufs=NPAIR) as wop,
            tc.tile_pool(name="outp_sb", bufs=3) as outsb,
        ):
            # Warm the PE clock (HAM) while the first input DMAs land
            # (~12us of dummy matmuls; vary operands/outputs so nothing
            # collapses into zero-duration issues).
            wu = singles.tile([128, 512], BF16, tag="warm")
            nc.vector.memset(wu, 0.0)
            wups = wops.tile([128, 512], F32, tag="wo", name="wupstile")
            for i in range(110):
                nc.tensor.matmul(
                    wups[:, (i % 2) * 256:(i % 2) * 256 + 256],
                    lhsT=wu[:, (i % 4) * 128:(i % 4) * 128 + 128],
                    rhs=wu[:, 0:256] if i % 2 == 0 else wu[:, 256:512],
                    start=True,
                    stop=True,
                )

            # ---- deferred-filler machinery: (due_chunk, fn) queue ----
            chunk_ctr = [0]
            pending = []

            def defer(margin, fn):
                pending.append([chunk_ctr[0] + margin, fn])

            def drain(limit=3):
                # strict FIFO: trace order IS the dependency order for
                # read-after-write pairs queued through here (e.g. norm_b
                # writes ctxT, wo_group reads it) — never reorder.
                n = 0
                while pending and pending[0][0] <= chunk_ctr[0] and n < limit:
                    pending.pop(0)[1]()
                    n += 1

            def drain_all():
                while pending:
                    pending.pop(0)[1]()

            # ---- projections ----
            def load_w(wdram, split=False):
                wt = wp.tile([128, DCH, HG], BF16, tag="w", name="wtile")
                if split:
                    nc.sync.dma_start(out=wt[:, :, 0:HG // 2], in_=wdram[:, :, 0:HG // 2])
                    nc.sync.dma_start(out=wt[:, :, HG // 2:HG], in_=wdram[:, :, HG // 2:HG])
                else:
                    nc.sync.dma_start(out=wt, in_=wdram)
                return wt

            def load_xts(xdram, t):
                xt = xtp.tile([128, DCH, 512], BF16, tag="xt", name="xtile")
                nc.sync.dma_start(out=xt, in_=xdram[:, t, :, :])
                return xt

            def proj_group_T(wt, xt, dst, m, t):
                # dst[m][:, t*512:+512] = (W.T chunk m).T @ xT
                ps = stp.tile([128, 512], F32, tag="st", name="psproj")
                for d in range(DCH):
                    nc.tensor.matmul(
                        ps,
                        lhsT=wt[:, d, m * 128:(m + 1) * 128],
                        rhs=xt[:, d, :],
                        start=(d == 0),
                        stop=(d == DCH - 1),
                    )
                nc.vector.tensor_copy(dst[m][:, t * 512:(t + 1) * 512], ps)

            def proj_group_V(wt, xt, t, s):
                # v_aug[t*4+s][:, h, 0:64] = (x @ Wv.T)[tok chunk, head h], masked
                t16 = t * 4 + s
                ps = stp.tile([128, 512], F32, tag="st", name="psv")
                for d in range(DCH):
                    nc.tensor.matmul(
                        ps,
                        lhsT=xt[:, d, s * 128:(s + 1) * 128],
                        rhs=wt[:, d, :],
                        start=(d == 0),
                        stop=(d == DCH - 1),
                    )
                nc.vector.tensor_scalar_mul(
                    vaug[t16][:, :, 0:DH],
                    ps.rearrange("p (h e) -> p h e", h=NH),
                    mc_sb[:, t16, 0:1],
                )
                nc.vector.tensor_copy(
                    vaug[t16][:, :, DH:DH + 1],
                    mc_sb[:, t16:t16 + 1, :],
                )

            wk_t = load_w(wk, split=True)
            wq_t = load_w(wq)
            wv_t = load_w(wv)

            # ---- wo output projection for one 128-token chunk, one half ----
            ot_tiles = {}
            wo_done = {}

            def wo_group(t16, oh, pool, tag):
                def fn():
                    ps = pool.tile([128, 512], F32, tag=tag, name="potile")
                    for c in range(NPAIR):
                        nc.tensor.matmul(
                            ps,
                            lhsT=ctxT[c][:, t16 * 128:(t16 + 1) * 128],
                            rhs=wo_sb[c][:, oh * 512:(oh + 1) * 512],
                            start=(c == 0),
                            stop=(c == NPAIR - 1),
                        )
                    ot = ot_tiles.get(t16)
                    if ot is None:
                        ot = outsb.tile([128, D], BF16, tag="ot", name="ottile")
                        ot_tiles[t16] = ot
                    nc.vector.tensor_copy(ot[:, oh * 512:(oh + 1) * 512], ps)
                    done = wo_done.setdefault(t16, [])
                    done.append(oh)
                    if len(done) == 2:
                        nc.sync.dma_start(
                            out=outp[t16 * 128:(t16 + 1) * 128, :], in_=ot
                        )
                        del ot_tiles[t16]
                return fn

            last_ex = [None]

            # ---- one attention sweep: q-block qb, head pair p ----
            def sweep(qb, p):
                nkc = 4 * (qb + 1)
                pv = [
                    pvp.tile([DH + 1, 512], F32, tag="pv", name="pvtile")
                    for _ in range(2)
                ]

                def issue_pv(kc, ex, off):
                    for ph in range(2):
                        nc.tensor.matmul(
                            pv[ph][:, off:512],
                            lhsT=vaug[kc][:, 2 * p + ph, :],
                            rhs=ex[:, ph, off:512],
                            start=(kc == 0),
                            stop=(kc == nkc - 1),
                        )

                prev = None
                last_ex[0] = None
                for kc in range(nkc):
                    j = kc - 4 * qb  # >=0 -> diagonal 512-block
                    off = j * 128 if j >= 0 else 0
                    st = stp.tile([128, 2, 512], F32, tag="st", name="sttile")
                    for ph in range(2):
                        nc.tensor.matmul(
                            st[:, ph, off:512],
                            lhsT=kT[p][ph * DH:(ph + 1) * DH,
                                       kc * 128:(kc + 1) * 128],
                            rhs=qT[p][ph * DH:(ph + 1) * DH,
                                      qb * 512 + off:(qb + 1) * 512],
                            start=True,
                            stop=True,
                        )
                    ex = expp.tile([128, 2, 512], BF16, tag="expst", name="extile")
                    nc.scalar.activation(
                        out=ex[:, :, off:512], in_=st[:, :, off:512], func=EXP
                    )
                    if j >= 0:
                        # causal mask: zero the strictly-upper part of the
                        # 128x128 diagonal block (0/1 multiply on DVE)
                        nc.vector.tensor_mul(
                            ex[:, :, off:off + 128],
                            ex[:, :, off:off + 128],
                            tri_sb,
                        )
                    if prev is not None:
                        issue_pv(*prev)
                    prev = (kc, ex, off)
                    last_ex[0] = ex
                    chunk_ctr[0] += 1
                    drain()
                issue_pv(*prev)

                # evacuate pv -> SBUF promptly (frees the 2 banks); move the
                # mask row to partition 0 with a tiny DMA for the reciprocal.
                pvs = [
                    pvsbp.tile([DH + 1, 512], F32, tag="pvs", name="pvstile")
                    for _ in range(2)
                ]
                ds = dsbp.tile([1, 2, 512], F32, tag="ds", name="dstile")
                if qb == TQ - 1 and p == NPAIR - 1:
                    # tail: denominator rows first so their DMA launches
                    # ~0.7us earlier; bulk evacuation follows
                    for ph in range(2):
                        nc.vector.tensor_copy(
                            pvs[ph][DH:DH + 1, :], pv[ph][DH:DH + 1, :]
                        )
                        nc.sync.dma_start(
                            out=ds[0:1, ph, :], in_=pvs[ph][DH:DH + 1, :]
                        )
                    for ph in range(2):
                        nc.vector.tensor_copy(pvs[ph][0:DH, :], pv[ph][0:DH, :])
                else:
                    for ph in range(2):
                        nc.vector.tensor_copy(pvs[ph], pv[ph])
                        nc.sync.dma_start(out=ds[0:1, ph, :], in_=pvs[ph][DH:DH + 1, :])

                bcs = [None, None]

                def norm_a():
                    rc = rcbp.tile([1, 2, 512], F32, tag="rc", name="rctile")
                    nc.vector.reciprocal_approx_fast(
                        out=rc[0:1, :, :], in_=ds[0:1, :, :]
                    )
                    for ph in range(2):
                        bcs[ph] = bcsp.tile([DH, 512], F32, tag="bcs", name="bcstile")
                        nc.gpsimd.partition_broadcast(
                            bcs[ph], rc[0:1, ph, :], channels=DH
                        )

                def norm_b():
                    nc.vector.tensor_mul(
                        ctxT[p][0:DH, qb * 512:(qb + 1) * 512],
                        pvs[0][0:DH, :],
                        bcs[0],
                    )
                    tb = tbp.tile([DH, 512], BF16, tag="tb", name="tbtile")
                    nc.vector.tensor_mul(tb, pvs[1][0:DH, :], bcs[1])
                    if qb == TQ - 1 and p == NPAIR - 1:
                        # tail: partition shift via PE identity matmul + DVE
                        # evac (a tiny SBUF->SBUF DMA costs ~2us of latency
                        # here and the idle re-throttles the PE clock)
                        ps = stp.tile([128, 512], F32, tag="st", name="shifttile")
                        nc.tensor.matmul(
                            ps[DH:128, :], lhsT=id_sb, rhs=tb, start=True, stop=True
                        )
                        nc.vector.tensor_copy(
                            ctxT[p][DH:128, qb * 512:(qb + 1) * 512], ps[DH:128, :]
                        )
                    else:
                        # partition shift rows 0-63 -> 64-127 via DMA
                        nc.sync.dma_start(
                            out=ctxT[p][DH:128, qb * 512:(qb + 1) * 512],
                            in_=tb,
                        )

                if qb == TQ - 1:
                    defer(1, norm_a)
                    defer(2, norm_b)
                else:
                    defer(2, norm_a)
                    defer(4, norm_b)

            # ---- main schedule ----
            xk0 = load_xts(xk, 0)
            for m in range(NPAIR):
                proj_group_T(wk_t, xk0, kT, m, 0)
            xq0 = load_xts(xq, 0)
            for m in range(NPAIR):
                proj_group_T(wq_t, xq0, qT, m, 0)
            xv0 = load_xts(xv, 0)
            wo_sb = [wop.tile([128, D], BF16, tag="wo", name="wotile") for _ in range(NPAIR)]
            for c in range(NPAIR):
                nc.sync.dma_start(out=wo_sb[c], in_=wo[c * 128:(c + 1) * 128, :])
            for s in range(4):
                defer(1 + s, (lambda s=s: proj_group_V(wv_t, xv0, 0, s)))

            for t in range(TQ):
                qb = t
                nchunks = 4 * 4 * (qb + 1)
                # queue next t-block's projections as fillers spread over
                # this block's sweeps (x DMAs issued inside the closures)
                if t + 1 < TQ:
                    xnext = {}
                    step = max(1, nchunks // 16)

                    def mk_load(xdram, key, t1=t + 1, xn=xnext):
                        def fn():
                            xn[key] = load_xts(xdram, t1)
                        return fn

                    def mk_T(wt, key, dst, m, t1=t + 1, xn=xnext):
                        def fn():
                            proj_group_T(wt, xn[key], dst, m, t1)
                        return fn

                    def mk_V(wt, key, s, t1=t + 1, xn=xnext):
                        def fn():
                            proj_group_V(wt, xn[key], t1, s)
                        return fn

                    fill = [mk_load(xk, "k")]
                    fill += [mk_T(wk_t, "k", kT, m) for m in range(NPAIR)]
                    fill += [mk_load(xq, "q")]
                    fill += [mk_T(wq_t, "q", qT, m) for m in range(NPAIR)]
                    fill += [mk_load(xv, "v")]
                    fill += [mk_V(wv_t, "v", s) for s in range(4)]
                    for idx, fn in enumerate(fill):
                        defer(1 + idx * step, fn)

                for p in range(NPAIR):
                    sweep(qb, p)

                # queue this q-block's output projection
                for i, t16 in enumerate(range(4 * qb, 4 * qb + 4)):
                    for oh in range(2):
                        if qb == TQ - 1:
                            pool, tag = [(wops, "wo"), (stp, "st")][(2 * i + oh) % 2]
                            due = 1 + i
                        else:
                            pool, tag = wops, "wo"
                            due = 2 + 2 * i + oh
                        defer(due, wo_group(t16, oh, pool, tag))

            # keep the PE clock warm across the final norm chain (the
            # ~5us idle otherwise re-throttles HAM and the tail wo matmuls
            # run at half clock); reading the last ex tile pins these to
            # the tail (they'd float to the front with no dependencies)
            kw = pvp.tile([DH + 1, 512], F32, tag="pv", name="kwtile")
            lex = last_ex[0]
            for i in range(56):
                nc.tensor.matmul(
                    kw[0:DH, (i % 2) * 256:(i % 2) * 256 + 256],
                    lhsT=wu[0:DH, 0:DH],
                    rhs=lex[0:DH, i % 2, 256:512],
                    start=True,
                    stop=True,
                )
            drain_all()

    nc.compile()
    return nc


_CACHE = {}


def _get_nc(L):
    if L not in _CACHE:
        _CACHE[L] = build(L)
    return _CACHE[L]


def make_in_maps(query, key, value, attention_mask, Wq, Wk, Wv, Wo):
    import ml_dtypes

    B, L, _ = query.shape
    scale = np.float32(1.0 / math.sqrt(DH))
    bf = lambda a: np.ascontiguousarray(np.asarray(a, np.float32)).astype(
        ml_dtypes.bfloat16
    )
    TQ, DCH = L // 512, D // 128
    # [p, t, d, l] = x.T[d*128+p, t*512+l] -> 8KB contiguous lines per block
    def relayout(x):
        xt = np.asarray(x, np.float32).T.reshape(DCH, 128, TQ, 512)
        return bf(np.ascontiguousarray(xt.transpose(1, 2, 0, 3)))

    xqT = [relayout(query[b]) for b in range(B)]
    xkT = [relayout(key[b]) for b in range(B)]
    xvT = [relayout(value[b]) for b in range(B)]
    kk, qq = np.meshgrid(np.arange(128), np.arange(128), indexing="ij")
    tri01 = np.where(kk <= qq, np.float32(1.0), np.float32(0.0)).astype(np.float32)
    tri2 = np.ascontiguousarray(np.concatenate([tri01, tri01], axis=1))
    in_maps = []
    for core in range(2 * B):
        b, hg = divmod(core, 2)
        sl = slice(hg * HG, (hg + 1) * HG)
        m2 = np.asarray(attention_mask[b]).astype(np.float32).reshape(-1, 128).T
        mc = np.ascontiguousarray(
            np.repeat(m2[:, :, None], NH, 2).reshape(128, -1), dtype=np.float32
        )
        def wrelayout(wt):
            # [p, d, :] = W_slice.T[d*128+p, :]
            return bf(np.ascontiguousarray(
                np.asarray(wt, np.float32).reshape(DCH, 128, HG).transpose(1, 0, 2)
            ))

        in_maps.append({
            "xq_r": xqT[b],
            "xk_r": xkT[b],
            "xv_r": xvT[b],
            "wq_r": wrelayout(np.asarray(Wq, np.float32)[sl, :].T * scale),
            "wk_r": wrelayout(np.asarray(Wk, np.float32)[sl, :].T),
            "wv_r": wrelayout(np.asarray(Wv, np.float32)[sl, :].T),
            "wo_t": bf(np.asarray(Wo, np.float32)[:, sl].T),
            "maskcol": mc,
            "trimask": bf(tri2),
            "ident64": bf(np.eye(DH, dtype=np.float32)),
        })
    return in_maps


def kernel(query, key, value, attention_mask, Wq, Wk, Wv, Wo, _res_hook=None):
    B, L, D_ = query.shape
    nc = _get_nc(L)
    in_maps = make_in_maps(query, key, value, attention_mask, Wq, Wk, Wv, Wo)
    res = bass_utils.run_bass_kernel_spmd(nc, in_maps, core_ids=list(range(8)))
    if _res_hook is not None:
        _res_hook(res)
    out = np.empty((B, L, D_), np.float32)
    for b in range(B):
        out[b] = res.results[2 * b]["outp"].astype(np.float32) + res.results[
            2 * b + 1
        ]["outp"].astype(np.float32)
    return out
